# revision 2
# baseline (speedup 1.0000x reference)
"""ColorHistogramLoss Trainium2 kernel v2 (8 NeuronCores, data-parallel).

Strategy: shard batch (32 -> 4 per core); 8 plane-triple iterations of
[128, 2048] f32 planes per core. Derived tiles are fp16 (2x stock DVE
throughput); all 27 histogram edge counts use dual-edge fused custom DVE
ops (2 counts packed per f32 accumulator) or ScalarE Sign passes.

Hue needs NO division/masks: piecewise hue is monotone in the chroma
angle atan2(g-b, 2r-g-b), so hue-edge counts are cross-multiply tests
y <> x*tan(theta_k). Sign-gating is done by pushing the out-of-halfplane
pixels +-1000 out of range (yp/ym tiles), making every test an ungated
dual custom op; complement counts are recovered on the host from
#{u>0} / #{u>=0} (two ScalarE Sign passes with -+1e-9 biases).
Sat edges use  mn > (1-c)*mx  (no d tile needed).
"""

import sys

if "/opt/trn_rl_repo" not in sys.path:
    sys.path.insert(0, "/opt/trn_rl_repo")

import numpy as np

from concourse import bacc, mybir, tile
from concourse import bass_utils

B, C, H, W = 32, 3, 512, 512
NCORES = 8
BPC = B // NCORES
P, F = 128, 2048
NITER = 2 * BPC
ACCW = 20
NPIX = B * H * W
ALPHA, BETA, GAMMA = 0.3, 0.4, 0.4

AF = mybir.AluOpType
F32 = mybir.dt.float32
F16 = mybir.dt.float16

LAST_EXEC_NS = None
_CACHE = {}

PACK = 4096.0
PUSH = 1000.0
EPS = 1e-9

# tan(theta_k) for hue edges k=1..4 (edges 6..9 share the same lines)
T12 = (0.42857142857142855, 1.6666666666666667)
T34 = (-1.6666666666666667, -0.42857142857142855)

# acc slot layout per iteration ([P,1] f32 accumulators):
#  0: W1 + PACK*W2      #{yp < x*t} duals        (hue 1,2 direct lt)
#  1: Q3 + PACK*Q4      #{yp <= x*t}             (hue 3,4 complement)
#  2: Q6 + PACK*Q7      #{ym > x*t}              (hue 6,7 complement)
#  3: G8 + PACK*G9      #{ym >= x*t}             (hue 8,9 direct ge)
#  4: sat 0.1 + PACK*0.2     (#{mn > (1-c)mx} = C_lt)
#  5: sat 0.3 + PACK*0.4
#  6: sat 0.5 + PACK*0.6
#  7: sat 0.7 + PACK*0.8
#  8: sat 0.9 + PACK*val#{mx<0.5}
#  9: sign(u - EPS)  -> #{u>0}, #{u<=0}
# 10: sign(u + EPS)  -> #{u>=0}, #{u<0}
# 11..18: val Sign sums for e in (.1,.2,.3,.4,.6,.7,.8,.9)
VAL_S_EDGES = (0.1, 0.2, 0.3, 0.4, 0.6, 0.7, 0.8, 0.9)


def _register_custom_ops():
    from concourse import dve_ops
    from concourse.dve_spec import C0, C1, C2, Spec, Src0, Src1, Zero, lower, _has_src1
    from concourse.dve_uop import DveOpSpec

    if hasattr(dve_ops, "DUAL_LT"):
        return dve_ops

    from operator import add as _add

    def _mk_ref(fn):
        def ref(in0, in1, c0, c1, c2):
            b = fn(
                in0.astype(np.float32),
                None if in1 is None else in1.astype(np.float32),
                c0, c1, c2,
            ).astype(np.float32)
            return b, b.reshape(b.shape[0], -1).sum(axis=-1, keepdims=True)
        return ref

    defs = [
        # ungated cross-mult duals on (x=Src0, y=Src1)
        ("DUAL_LT",
         Spec(body=(Src1 < Src0 * C0) + C2 * (Src1 < Src0 * C1),
              accum=_add, accum_init=Zero,
              reference=_mk_ref(lambda x, y, c0, c1, c2:
                  (y < x * c0) + c2 * (y < x * c1)))),
        ("DUAL_LE",
         Spec(body=(Src1 <= Src0 * C0) + C2 * (Src1 <= Src0 * C1),
              accum=_add, accum_init=Zero,
              reference=_mk_ref(lambda x, y, c0, c1, c2:
                  (y <= x * c0) + c2 * (y <= x * c1)))),
        ("DUAL_GT",
         Spec(body=(Src1 > Src0 * C0) + C2 * (Src1 > Src0 * C1),
              accum=_add, accum_init=Zero,
              reference=_mk_ref(lambda x, y, c0, c1, c2:
                  (y > x * c0) + c2 * (y > x * c1)))),
        ("DUAL_GE",
         Spec(body=(Src1 >= Src0 * C0) + C2 * (Src1 >= Src0 * C1),
              accum=_add, accum_init=Zero,
              reference=_mk_ref(lambda x, y, c0, c1, c2:
                  (y >= x * c0) + c2 * (y >= x * c1)))),
        # single-src dual: #{x < c0} + PACK*#{x < c1}
        ("DUAL_VAL",
         Spec(body=(Src0 < C0) + C2 * (Src0 < C1),
              accum=_add, accum_init=Zero,
              reference=_mk_ref(lambda x, y, c0, c1, c2:
                  (x < c0) + c2 * (x < c1)))),
        # sat edge + val edge mixed: #{mn > c0*mx} + PACK*#{mx < c1}
        ("SATVALM",
         Spec(body=(Src1 > Src0 * C0) + C2 * (Src0 < C1),
              accum=_add, accum_init=Zero,
              reference=_mk_ref(lambda x, y, c0, c1, c2:
                  (y > x * c0) + c2 * (x < c1)))),
    ]
    for name, spec in defs:
        row = 1 + len(dve_ops.OPS)
        shas = {}
        for ver in ("v3", "v4"):
            uops = lower(spec, ver=ver)
            shas[ver] = DveOpSpec(
                name=name, opcode=row, uops=uops, rd1_en=_has_src1(spec)
            ).sha(ver)
        op = dve_ops.DveOp(name, spec, False, uops_sha=shas)
        dve_ops.OPS.append(op)
        dve_ops.CUSTOM_DVE_SPECS[name] = spec
        dve_ops._SUB_OPCODE_FOR_NAME[name] = row
        setattr(dve_ops, name, op)
    return dve_ops


def _build():
    dve_ops = _register_custom_ops()
    nc = bacc.Bacc(
        "TRN2", target_bir_lowering=False, debug=False, num_devices=NCORES
    )
    xr = nc.dram_tensor("x_real", [BPC * C * P, F], F32, kind="ExternalInput").ap()
    xf = nc.dram_tensor("x_fake", [BPC * C * P, F], F32, kind="ExternalInput").ap()
    out = nc.dram_tensor("out", [NITER * P, ACCW], F32, kind="ExternalOutput").ap()

    SIGN = mybir.ActivationFunctionType.Sign
    COPY = mybir.ActivationFunctionType.Copy

    with tile.TileContext(nc) as tc:
        with tc.tile_pool(name="main", bufs=2) as io_pool, tc.tile_pool(
            name="tmp", bufs=1
        ) as tmp_pool:
            # bias tiles for ScalarE Sign passes
            ubias_m = tmp_pool.tile([P, 1], F32, tag="ubm", name="ubm")
            nc.gpsimd.memset(ubias_m[:], -EPS)
            ubias_p = tmp_pool.tile([P, 1], F32, tag="ubp", name="ubp")
            nc.gpsimd.memset(ubias_p[:], EPS)
            vbias = []
            for idx, e in enumerate(VAL_S_EDGES):
                bt = tmp_pool.tile([P, 1], F32, tag=f"vb{idx}", name=f"vb{idx}")
                nc.gpsimd.memset(bt[:], -e)
                vbias.append(bt)

            for it in range(NITER):
                src = xr if it < BPC else xf
                bi = it % BPC

                def plane(c):
                    q = bi * C + c
                    return src[q * P : (q + 1) * P, :]

                r32 = io_pool.tile([P, F], F32, tag="r32")
                g32 = io_pool.tile([P, F], F32, tag="g32")
                b32 = io_pool.tile([P, F], F32, tag="b32")
                nc.sync.dma_start(g32[:], plane(1))
                nc.sync.dma_start(b32[:], plane(2))
                nc.sync.dma_start(r32[:], plane(0))

                # fp16 casts: r,g on ScalarE; b on VectorE (double-buffered
                # across iterations to break WAR stalls on cross-engine reads)
                g16 = tmp_pool.tile([P, F], F16, tag="g16", name="g16", bufs=2)
                r16 = tmp_pool.tile([P, F], F16, tag="r16", name="r16", bufs=2)
                b16 = tmp_pool.tile([P, F], F16, tag="b16", name="b16", bufs=2)
                nc.vector.tensor_copy(b16[:], b32[:])
                nc.scalar.activation(g16[:], g32[:], COPY)
                nc.scalar.activation(r16[:], r32[:], COPY)

                V = nc.vector
                u = tmp_pool.tile([P, F], F16, tag="u", name="u", bufs=2)
                m = tmp_pool.tile([P, F], F16, tag="m", name="m")
                t2 = tmp_pool.tile([P, F], F16, tag="t2", name="t2")
                yp = tmp_pool.tile([P, F], F16, tag="yp", name="yp")
                ym = tmp_pool.tile([P, F], F16, tag="ym", name="ym")
                t = tmp_pool.tile([P, F], F16, tag="t", name="t")
                r2 = tmp_pool.tile([P, F], F16, tag="r2", name="r2")
                x = tmp_pool.tile([P, F], F16, tag="x", name="x")
                m1 = tmp_pool.tile([P, F], F16, tag="m1", name="m1")
                mx = tmp_pool.tile([P, F], F16, tag="mx", name="mx", bufs=2)
                n1 = tmp_pool.tile([P, F], F16, tag="n1", name="n1")
                mn = tmp_pool.tile([P, F], F16, tag="mn", name="mn")
                scr = tmp_pool.tile([P, F], F16, tag="scr", name="scr")
                scr2 = tmp_pool.tile([P, F], F16, tag="scr2", name="scr2")
                acc = io_pool.tile([P, ACCW], F32, tag="acc")

                V.tensor_tensor(u[:], g16[:], b16[:], AF.subtract)
                # push tiles: m = 1000*[u<0]; yp = u + m; ym = u + (m-1000)
                V.tensor_scalar(m[:], u[:], 0.0, PUSH, AF.is_lt, AF.mult)
                V.tensor_scalar(t2[:], m[:], PUSH, None, AF.subtract)
                V.tensor_tensor(yp[:], u[:], m[:], AF.add)
                V.tensor_tensor(ym[:], u[:], t2[:], AF.add)
                V.tensor_tensor(t[:], g16[:], b16[:], AF.add)
                V.tensor_scalar(r2[:], r16[:], 2.0, None, AF.mult)
                V.tensor_tensor(x[:], r2[:], t[:], AF.subtract)
                V.tensor_tensor(m1[:], r16[:], g16[:], AF.max)
                V.tensor_tensor(mx[:], m1[:], b16[:], AF.max)
                V.tensor_tensor(n1[:], r16[:], g16[:], AF.min)
                V.tensor_tensor(mn[:], n1[:], b16[:], AF.min)

                # hue duals
                V._custom_dve(dve_ops.DUAL_LT, out=scr[:], in0=x[:], in1=yp[:],
                              s0=T12[0], s1=T12[1], imm2=PACK,
                              accum_out=acc[:, 0:1])
                V._custom_dve(dve_ops.DUAL_LE, out=scr[:], in0=x[:], in1=yp[:],
                              s0=T34[0], s1=T34[1], imm2=PACK,
                              accum_out=acc[:, 1:2])
                V._custom_dve(dve_ops.DUAL_GT, out=scr[:], in0=x[:], in1=ym[:],
                              s0=T12[0], s1=T12[1], imm2=PACK,
                              accum_out=acc[:, 2:3])
                V._custom_dve(dve_ops.DUAL_GE, out=scr[:], in0=x[:], in1=ym[:],
                              s0=T34[0], s1=T34[1], imm2=PACK,
                              accum_out=acc[:, 3:4])
                # sat duals on (mx, mn): edge c -> #{mn > (1-c)*mx}
                for j in range(4):
                    c_lo, c_hi = 0.1 * (2 * j + 1), 0.1 * (2 * j + 2)
                    V._custom_dve(dve_ops.DUAL_GT, out=scr[:], in0=mx[:], in1=mn[:],
                                  s0=1.0 - c_lo, s1=1.0 - c_hi, imm2=PACK,
                                  accum_out=acc[:, 4 + j : 5 + j])
                V._custom_dve(dve_ops.SATVALM, out=scr[:], in0=mx[:], in1=mn[:],
                              s0=1.0 - 0.9, s1=0.5, imm2=PACK,
                              accum_out=acc[:, 8:9])

                # ScalarE: u sign gates + 8 val Sign passes
                nc.scalar.activation(scr2[:], u[:], SIGN, bias=ubias_m[:],
                                     accum_out=acc[:, 9:10])
                nc.scalar.activation(scr2[:], u[:], SIGN, bias=ubias_p[:],
                                     accum_out=acc[:, 10:11])
                for idx in range(8):
                    nc.scalar.activation(
                        scr2[:], mx[:], SIGN, bias=vbias[idx][:],
                        accum_out=acc[:, 11 + idx : 12 + idx],
                    )

                nc.sync.dma_start(out[it * P : (it + 1) * P, :], acc[:, :])

    nc.compile()
    return nc


def _register_ntff_hook():
    import types

    import antenv

    if "antenv.axon_hooks" not in sys.modules:
        mod = types.ModuleType("antenv.axon_hooks")
        holder = [None]
        mod.set_axon_ntff_profile_hook = lambda h: holder.__setitem__(0, h)
        mod.get_axon_ntff_profile_hook = lambda: holder[0]
        sys.modules["antenv.axon_hooks"] = mod
        antenv.axon_hooks = mod
    from antenv import axon_hooks

    if axon_hooks.get_axon_ntff_profile_hook() is None:
        from trn_agent_boot.trn_boot import _ntff_profile_via_ctypes

        axon_hooks.set_axon_ntff_profile_hook(
            _ntff_profile_via_ctypes("/opt/axon/libaxon_pjrt.so")
        )
    bass_utils.upload_artifacts = lambda tmpdir: tmpdir


def _get_nc():
    if "nc" not in _CACHE:
        _CACHE["nc"] = _build()
    return _CACHE["nc"]


def kernel(x_real: np.ndarray, x_fake: np.ndarray) -> np.ndarray:
    global LAST_EXEC_NS
    nc = _get_nc()

    in_maps = []
    for c in range(NCORES):
        sl = slice(c * BPC, (c + 1) * BPC)
        in_maps.append(
            {
                "x_real": np.ascontiguousarray(x_real[sl]).reshape(BPC * C * P, F),
                "x_fake": np.ascontiguousarray(x_fake[sl]).reshape(BPC * C * P, F),
            }
        )

    import os

    trace = bool(int(os.environ.get("KERNEL_TRACE", "0")))
    if trace:
        _register_ntff_hook()
    res = bass_utils.run_bass_kernel_spmd(
        nc, in_maps, core_ids=list(range(NCORES)), trace=trace
    )
    LAST_EXEC_NS = res.exec_time_ns
    _CACHE["last_res"] = res

    IPACK = int(PACK)
    C_lt = np.zeros((2, 3, 9), np.float64)
    W12 = np.zeros((2, 2)); Q34 = np.zeros((2, 2))
    Q67 = np.zeros((2, 2)); G89 = np.zeros((2, 2))
    sat9 = np.zeros(2); val5 = np.zeros(2)
    sgn_m = np.zeros(2); sgn_p = np.zeros(2)
    sign_sums = np.zeros((2, 8))
    last_le0 = np.zeros(1); last_neg = np.zeros(1); last_val = np.zeros(8)
    for core_out in res.results:
        o = np.asarray(core_out["out"]).reshape(NITER, P, ACCW).astype(np.float64)
        for t_idx, sl in ((0, slice(0, BPC)), (1, slice(BPC, NITER))):
            blk = o[sl]
            tot = blk.sum(axis=(0, 1))
            for arr, slot in ((W12, 0), (Q34, 1), (Q67, 2), (G89, 3)):
                p = blk[:, :, slot].astype(np.int64)
                arr[t_idx, 0] += (p % IPACK).sum()
                arr[t_idx, 1] += (p // IPACK).sum()
            for j in range(4):
                p = blk[:, :, 4 + j].astype(np.int64)
                C_lt[t_idx, 1, 2 * j] += (p % IPACK).sum()
                C_lt[t_idx, 1, 2 * j + 1] += (p // IPACK).sum()
            p8 = blk[:, :, 8].astype(np.int64)
            sat9[t_idx] += (p8 % IPACK).sum()
            val5[t_idx] += (p8 // IPACK).sum()
            sgn_m[t_idx] += tot[9]
            sgn_p[t_idx] += tot[10]
            sign_sums[t_idx] += tot[11:19]
    # sign gate decode: sgn_m = #{u>0} - #{u<=0}; sgn_p = #{u>=0} - #{u<0}
    # fake tensor: sign sums cover 3 of 4 iters; last iter counted directly
    NP34 = NPIX * (NITER // 2 - 1) / (NITER // 2)   # pixels in sign-iters
    NPL = NPIX // (NITER // 2)                      # pixels in last iter
    n_pos = (NPIX + sgn_m) / 2.0
    n_pos0 = (NPIX + sgn_p) / 2.0
    n_neg = NPIX - n_pos0             # #{u<0}
    # hue cumulative counts
    C_lt[:, 0, 0] = W12[:, 0]
    C_lt[:, 0, 1] = W12[:, 1]
    C_lt[:, 0, 2] = n_pos0 - Q34[:, 0]
    C_lt[:, 0, 3] = n_pos0 - Q34[:, 1]
    C_lt[:, 0, 4] = n_pos                       # edge 3.0 (theta=pi)
    C_lt[:, 0, 5] = NPIX - (n_neg - Q67[:, 0])  # N - G6
    C_lt[:, 0, 6] = NPIX - (n_neg - Q67[:, 1])
    C_lt[:, 0, 7] = NPIX - G89[:, 0]
    C_lt[:, 0, 8] = NPIX - G89[:, 1]
    C_lt[:, 1, 8] = sat9
    # val: Sign decode (fake: 3 sign-iters + last-iter direct counts)
    val_lt = (NPIX - sign_sums) / 2.0
    for i, e in enumerate(VAL_S_EDGES):
        C_lt[:, 2, int(round(e * 10)) - 1] = val_lt[:, i]
    C_lt[:, 2, 4] = val5

    hist = np.zeros((2, 3, 10), np.float64)
    hist[:, :, 0] = C_lt[:, :, 0]
    hist[:, :, 1:9] = C_lt[:, :, 1:] - C_lt[:, :, :-1]
    hist[:, :, 9] = NPIX - C_lt[:, :, 8]

    dmean = np.abs(hist[0] - hist[1]).mean(axis=1)
    loss = ALPHA * dmean[0] + BETA * dmean[1] + GAMMA * dmean[2]
    return np.asarray(loss, dtype=np.float32)
